# revision 18
# baseline (speedup 1.0000x reference)
"""NTM layer kernel for Trainium2 (8 NeuronCores, batch-parallel).

Full inputs in, full outputs out. Shards batch dim B=8192 across 8 cores
(1024 rows each), replicates the small controller/head weights.
"""

import numpy as np

import concourse.bass as bass
import concourse.bacc as bacc
import concourse.tile as tile
from concourse import masks, mybir
from concourse.bass_utils import run_bass_kernel_spmd

F32 = mybir.dt.float32
AF = mybir.ActivationFunctionType
AX = mybir.AxisListType
OP = mybir.AluOpType

B, D, L, C, H = 8192, 256, 128, 64, 100
N_CORES = 8
BC = B // N_CORES  # rows per core
P = 128            # batch rows per tile (SBUF partitions)
NT = BC // P       # tiles per core
EPS = 1e-12

WEIGHT_SPECS = [
    ("W1", [D + C, H]), ("b1", [H]), ("W2", [H, H]), ("b2", [H]),
    ("Wk_r", [H, C]), ("bk_r", [C]), ("Ws_r", [H, 1]), ("bs_r", [1]),
    ("Wk_w", [H, C]), ("bk_w", [C]), ("Ws_w", [H, 1]), ("bs_w", [1]),
    ("We_w", [H, C]), ("be_w", [C]), ("Wa_w", [H, C]), ("ba_w", [C]),
]

# column layout of the fused heads matmul output
KR0, SR0, KW0, SW0, E0, A0, NHEAD = 0, 64, 65, 129, 130, 194, 258


def _build_program():
    nc = bacc.Bacc()

    inputs_h = nc.declare_dram_parameter("inputs", [BC, D], F32, isOutput=False)
    memory_h = nc.declare_dram_parameter("memory", [BC, L, C], F32, isOutput=False)
    reading_h = nc.declare_dram_parameter("reading", [BC, 1, C], F32, isOutput=False)
    wh = {
        name: nc.declare_dram_parameter(name, shape, F32, isOutput=False)
        for name, shape in WEIGHT_SPECS
    }
    read_h = nc.declare_dram_parameter("read_out", [BC, C], F32, isOutput=True)
    newmem_h = nc.declare_dram_parameter("new_mem", [BC, L, C], F32, isOutput=True)

    inputs_ap = inputs_h[:]
    memory_ap = memory_h[:]
    reading_ap = reading_h[:]
    read_ap = read_h[:]
    newmem_ap = newmem_h[:]

    LH = L // 2  # process big [P, L, C] passes in L-halves (16KB/partition)

    with tile.TileContext(nc) as tc:
        with (
            tc.tile_pool(name="singles", bufs=1) as singles,
            tc.tile_pool(name="mem_pool", bufs=2) as mem_pool,
            tc.tile_pool(name="bsc", bufs=2) as bsc,      # shared big scratch ring
            tc.tile_pool(name="wa", bufs=2) as wa,        # W / A outer products
            tc.tile_pool(name="outp", bufs=2) as outp,    # new_mem halves out
            tc.tile_pool(name="ctl", bufs=2) as ctl,
            tc.tile_pool(name="ps_tr", bufs=3, space="PSUM") as ps_tr,
            tc.tile_pool(name="ps_mm", bufs=2, space="PSUM") as ps_mm,
            tc.tile_pool(name="ps_hd", bufs=2, space="PSUM") as ps_hd,
        ):
            # ---- one-time weight prep ----
            ident = singles.tile([P, P], F32)
            masks.make_identity(nc, ident)

            w1a = singles.tile([128, H], F32)
            w1b = singles.tile([128, H], F32)
            w1c = singles.tile([64, H], F32)
            nc.sync.dma_start(out=w1a, in_=wh["W1"][:][0:128, :])
            nc.sync.dma_start(out=w1b, in_=wh["W1"][:][128:256, :])
            nc.sync.dma_start(out=w1c, in_=wh["W1"][:][256:320, :])
            w2_sb = singles.tile([H, H], F32)
            nc.sync.dma_start(out=w2_sb, in_=wh["W2"][:])
            b1_sb = singles.tile([H, 1], F32)
            nc.sync.dma_start(out=b1_sb, in_=wh["b1"][:].unsqueeze(1))
            b2_sb = singles.tile([H, 1], F32)
            nc.sync.dma_start(out=b2_sb, in_=wh["b2"][:].unsqueeze(1))

            rhs_sb = singles.tile([H, NHEAD], F32)
            bias_row = singles.tile([1, NHEAD], F32)
            for w_name, b_name, c0, c1 in [
                ("Wk_r", "bk_r", KR0, SR0),
                ("Ws_r", "bs_r", SR0, KW0),
                ("Wk_w", "bk_w", KW0, SW0),
                ("Ws_w", "bs_w", SW0, E0),
                ("We_w", "be_w", E0, A0),
                ("Wa_w", "ba_w", A0, NHEAD),
            ]:
                nc.sync.dma_start(out=rhs_sb[:, c0:c1], in_=wh[w_name][:])
                nc.sync.dma_start(
                    out=bias_row[:, c0:c1], in_=wh[b_name][:].unsqueeze(0)
                )
            ones_sb = singles.tile([1, P], F32)
            nc.vector.memset(ones_sb, 1.0)

            # ---- per-tile pipeline ----
            for t in range(NT):
                r0 = t * P

                feats = ctl.tile([P, D + C], F32)
                nc.scalar.dma_start(out=feats[:, 0:D], in_=inputs_ap[r0:r0 + P, :])
                nc.scalar.dma_start(
                    out=feats[:, D:D + C].unsqueeze(1), in_=reading_ap[r0:r0 + P]
                )
                m = mem_pool.tile([P, L, C], F32)
                nc.sync.dma_start(out=m, in_=memory_ap[r0:r0 + P])

                # controller: h1 = tanh(feats@W1+b1); ctr = tanh(h1@W2+b2)
                t0 = ps_tr.tile([P, P], F32, tag="tr")
                nc.tensor.transpose(t0, feats[:, 0:128], ident)
                t1 = ps_tr.tile([P, P], F32, tag="tr")
                nc.tensor.transpose(t1, feats[:, 128:256], ident)
                t2 = ps_tr.tile([64, P], F32, tag="tr")
                nc.tensor.transpose(t2, feats[:, 256:320], ident)
                fT0 = ctl.tile([P, P], F32)
                nc.scalar.copy(fT0, t0)
                fT1 = ctl.tile([P, P], F32)
                nc.scalar.copy(fT1, t1)
                fT2 = ctl.tile([64, P], F32)
                nc.scalar.copy(fT2, t2)

                h1p = ps_mm.tile([H, P], F32, tag="mm")
                nc.tensor.matmul(h1p, lhsT=w1a, rhs=fT0, start=True, stop=False)
                nc.tensor.matmul(h1p, lhsT=w1b, rhs=fT1, start=False, stop=False)
                nc.tensor.matmul(h1p, lhsT=w1c, rhs=fT2, start=False, stop=True)
                h1T = ctl.tile([H, P], F32)
                nc.scalar.activation(h1T, h1p, AF.Tanh, bias=b1_sb)

                ctrp = ps_mm.tile([H, P], F32, tag="mm")
                nc.tensor.matmul(ctrp, lhsT=w2_sb, rhs=h1T, start=True, stop=True)
                ctrT = ctl.tile([H, P], F32)
                nc.scalar.activation(ctrT, ctrp, AF.Tanh, bias=b2_sb)

                # all six head projections; biases via 1-partition ones matmul
                hd = ps_hd.tile([P, NHEAD], F32)
                nc.tensor.matmul(hd, lhsT=ctrT, rhs=rhs_sb, start=True, stop=False)
                nc.tensor.matmul(hd, lhsT=ones_sb, rhs=bias_row, start=False, stop=True)

                # memory Frobenius norm^2 per batch row (ACT passes, halves)
                nsqa = bsc.tile([P, LH, C], F32, tag="bs")
                nm2a = ctl.tile([P, 1], F32)
                nc.scalar.activation(nsqa, m[:, 0:LH, :], AF.Square, accum_out=nm2a)
                nsqb = bsc.tile([P, LH, C], F32, tag="bs")
                nm2b = ctl.tile([P, 1], F32)
                nc.scalar.activation(nsqb, m[:, LH:L, :], AF.Square, accum_out=nm2b)
                nm2 = ctl.tile([P, 1], F32)
                nc.vector.tensor_add(nm2, nm2a, nm2b)
                nm = ctl.tile([P, 1], F32)
                nc.scalar.activation(nm, nm2, AF.Sqrt)
                nc.vector.tensor_scalar_max(nm, nm, EPS)

                # sigmoids grouped (single ACT table switch)
                s_r = ctl.tile([P, 1], F32)
                nc.scalar.activation(s_r, hd[:, SR0:SR0 + 1], AF.Sigmoid)
                s_w = ctl.tile([P, 1], F32)
                nc.scalar.activation(s_w, hd[:, SW0:SW0 + 1], AF.Sigmoid)
                e_sb = ctl.tile([P, C], F32)
                nc.scalar.activation(e_sb, hd[:, E0:A0], AF.Sigmoid)
                a_sb = ctl.tile([P, C], F32)
                nc.scalar.activation(a_sb, hd[:, A0:NHEAD], AF.Sigmoid)

                # per-head addressing: alpha = s / (max(|k|,eps)*max(|M|_F,eps))
                rsums = {}
                for name, k0, s_sig in (("r", KR0, s_r), ("w", KW0, s_w)):
                    k_ap = hd[:, k0:k0 + C]
                    ksq = ctl.tile([P, C], F32)
                    nk2 = ctl.tile([P, 1], F32)
                    nc.scalar.activation(ksq, k_ap, AF.Square, accum_out=nk2)
                    nk = ctl.tile([P, 1], F32)
                    nc.scalar.activation(nk, nk2, AF.Sqrt)
                    nc.vector.tensor_scalar_max(nk, nk, EPS)
                    den = ctl.tile([P, 1], F32)
                    nc.vector.tensor_mul(den, nk, nm)
                    rden = ctl.tile([P, 1], F32)
                    nc.vector.reciprocal(rden, den)
                    alpha = ctl.tile([P, 1], F32)
                    nc.vector.tensor_mul(alpha, s_sig, rden)

                    # logits: K[b,l] = alpha[b] * sum_c M[b,l,c]*k[b,c]
                    Kl = ctl.tile([P, L], F32)
                    kb = k_ap.unsqueeze(1).broadcast_to([P, LH, C])
                    for lh in range(2):
                        l0 = lh * LH
                        prod = bsc.tile([P, LH, C], F32, tag="bs")
                        nc.vector.scalar_tensor_tensor(
                            out=prod, in0=m[:, l0:l0 + LH, :], scalar=alpha,
                            in1=kb, op0=OP.mult, op1=OP.mult,
                        )
                        nc.vector.tensor_reduce(
                            Kl[:, l0:l0 + LH], prod, axis=AX.X, op=OP.add
                        )

                    # softmax over l: exp(K - max), sum via ACT accumulate
                    mxn = ctl.tile([P, 1], F32)
                    nc.vector.tensor_reduce(
                        mxn, Kl, axis=AX.X, op=OP.max, negate=True
                    )
                    ex = ctl.tile([P, L], F32)
                    sume = ctl.tile([P, 1], F32)
                    nc.scalar.activation(ex, Kl, AF.Exp, bias=mxn, accum_out=sume)
                    rsum = ctl.tile([P, 1], F32)
                    nc.vector.reciprocal(rsum, sume)
                    rsums[name] = (ex, rsum)

                # read output: read[b,c] = sum_l M[b,l,c] * w_r[b,l]
                ex_r, rsum_r = rsums["r"]
                rh = []
                for lh in range(2):
                    l0 = lh * LH
                    prodRT = bsc.tile([P, C, LH], F32, tag="bs")
                    mT = m[:, l0:l0 + LH, :].transpose([0, 2, 1])
                    exb = ex_r[:, l0:l0 + LH].unsqueeze(1).broadcast_to([P, C, LH])
                    nc.vector.scalar_tensor_tensor(
                        out=prodRT, in0=mT, scalar=rsum_r, in1=exb,
                        op0=OP.mult, op1=OP.mult,
                    )
                    rp = ctl.tile([P, C], F32, tag=f"rh{lh}")
                    nc.vector.tensor_reduce(rp, prodRT, axis=AX.X, op=OP.add)
                    rh.append(rp)
                read_sb = ctl.tile([P, C], F32)
                nc.vector.tensor_add(read_sb, rh[0], rh[1])
                nc.scalar.dma_start(out=read_ap[r0:r0 + P, :], in_=read_sb)

                # memory update: new_mem = M - M*(w_w x e) + (w_w x a)
                ex_w, rsum_w = rsums["w"]
                wwn = ctl.tile([P, L], F32)
                nc.vector.tensor_scalar_mul(wwn, ex_w, rsum_w)
                eb = e_sb.unsqueeze(1).broadcast_to([P, LH, C])
                ab = a_sb.unsqueeze(1).broadcast_to([P, LH, C])
                for lh in range(2):
                    l0 = lh * LH
                    msl = m[:, l0:l0 + LH, :]
                    wwb = wwn[:, l0:l0 + LH].unsqueeze(2).broadcast_to([P, LH, C])
                    Wh = wa.tile([P, LH, C], F32, tag="wa")
                    nc.gpsimd.tensor_tensor(out=Wh, in0=wwb, in1=eb, op=OP.mult)
                    Yh = bsc.tile([P, LH, C], F32, tag="bs")
                    nc.vector.scalar_tensor_tensor(
                        out=Yh, in0=Wh, scalar=1.0, in1=msl,
                        op0=OP.subtract, op1=OP.mult,
                    )  # (W-1)*M
                    Ah = wa.tile([P, LH, C], F32, tag="wa")
                    nc.gpsimd.tensor_tensor(out=Ah, in0=wwb, in1=ab, op=OP.mult)
                    Oh = outp.tile([P, LH, C], F32)
                    nc.vector.tensor_sub(Oh, Ah, Yh)  # A - (W-1)*M
                    eng = nc.sync if lh == 0 else nc.scalar
                    eng.dma_start(
                        out=newmem_ap[r0:r0 + P, l0:l0 + LH, :], in_=Oh
                    )

    return nc


_NC = None


def _get_program():
    global _NC
    if _NC is None:
        _NC = _build_program()
        _NC.finalize()
    return _NC


def _in_maps(inputs):
    weights = {name: np.ascontiguousarray(inputs[name], dtype=np.float32)
               for name, _ in WEIGHT_SPECS}
    in_maps = []
    for i in range(N_CORES):
        sl = slice(i * BC, (i + 1) * BC)
        in_maps.append({
            "inputs": np.ascontiguousarray(inputs["inputs"][sl], dtype=np.float32),
            "memory": np.ascontiguousarray(inputs["memory"][sl], dtype=np.float32),
            "reading": np.ascontiguousarray(inputs["reading"][sl], dtype=np.float32),
            **weights,
        })
    return in_maps


def _run(inputs, trace=False):
    nc = _get_program()
    return run_bass_kernel_spmd(nc, _in_maps(inputs), list(range(N_CORES)), trace=trace)


def kernel(**inputs):
    res = _run(inputs).results
    read = np.concatenate([res[i]["read_out"] for i in range(N_CORES)], axis=0)
    new_mem = np.concatenate([res[i]["new_mem"] for i in range(N_CORES)], axis=0)
    return read, new_mem


# revision 21
# speedup vs baseline: 8.7609x; 8.7609x over previous
"""NTM layer kernel for Trainium2 (8 NeuronCores, batch-parallel).

Full inputs in, full outputs out. Shards batch dim B=8192 across 8 cores
(1024 rows each), replicates the small controller/head weights.
"""

import numpy as np

import concourse.bass as bass
import concourse.bacc as bacc
import concourse.tile as tile
from concourse import masks, mybir
from concourse.bass_utils import run_bass_kernel_spmd

F32 = mybir.dt.float32
AF = mybir.ActivationFunctionType
AX = mybir.AxisListType
OP = mybir.AluOpType

B, D, L, C, H = 8192, 256, 128, 64, 100
N_CORES = 8
BC = B // N_CORES  # rows per core
P = 128            # batch rows per tile (SBUF partitions)
NT = BC // P       # tiles per core
EPS = 1e-12

WEIGHT_SPECS = [
    ("W1", [D + C, H]), ("b1", [H]), ("W2", [H, H]), ("b2", [H]),
    ("Wk_r", [H, C]), ("bk_r", [C]), ("Ws_r", [H, 1]), ("bs_r", [1]),
    ("Wk_w", [H, C]), ("bk_w", [C]), ("Ws_w", [H, 1]), ("bs_w", [1]),
    ("We_w", [H, C]), ("be_w", [C]), ("Wa_w", [H, C]), ("ba_w", [C]),
]

# column layout of the fused heads matmul output
KR0, SR0, KW0, SW0, E0, A0, NHEAD = 0, 64, 65, 129, 130, 194, 258


def _build_program(n_repeat=1):
    nc = bacc.Bacc()

    inputs_h = nc.declare_dram_parameter("inputs", [BC, D], F32, isOutput=False)
    memory_h = nc.declare_dram_parameter("memory", [BC, L, C], F32, isOutput=False)
    reading_h = nc.declare_dram_parameter("reading", [BC, 1, C], F32, isOutput=False)
    wh = {
        name: nc.declare_dram_parameter(name, shape, F32, isOutput=False)
        for name, shape in WEIGHT_SPECS
    }
    read_h = nc.declare_dram_parameter("read_out", [BC, C], F32, isOutput=True)
    newmem_h = nc.declare_dram_parameter("new_mem", [BC, L, C], F32, isOutput=True)

    inputs_ap = inputs_h[:]
    memory_ap = memory_h[:]
    reading_ap = reading_h[:]
    read_ap = read_h[:]
    newmem_ap = newmem_h[:]

    LH = L // 2  # process big [P, L, C] passes in L-halves (16KB/partition)

    with tile.TileContext(nc) as tc:
        with (
            tc.tile_pool(name="singles", bufs=1) as singles,
            tc.tile_pool(name="mem_pool", bufs=2) as mem_pool,
            tc.tile_pool(name="bsc", bufs=2) as bsc,      # shared big scratch ring
            tc.tile_pool(name="wa", bufs=2) as wa,        # W / A outer products
            tc.tile_pool(name="outp", bufs=2) as outp,    # new_mem halves out
            tc.tile_pool(name="ctl", bufs=2) as ctl,
            tc.tile_pool(name="ps_tr", bufs=3, space="PSUM") as ps_tr,
            tc.tile_pool(name="ps_mm", bufs=2, space="PSUM") as ps_mm,
            tc.tile_pool(name="ps_hd", bufs=2, space="PSUM") as ps_hd,
        ):
            # ---- one-time weight prep ----
            ident = singles.tile([P, P], F32)
            masks.make_identity(nc, ident)

            w1a = singles.tile([128, H], F32)
            w1b = singles.tile([128, H], F32)
            w1c = singles.tile([64, H], F32)
            nc.sync.dma_start(out=w1a, in_=wh["W1"][:][0:128, :])
            nc.sync.dma_start(out=w1b, in_=wh["W1"][:][128:256, :])
            nc.sync.dma_start(out=w1c, in_=wh["W1"][:][256:320, :])
            w2_sb = singles.tile([H, H], F32)
            nc.sync.dma_start(out=w2_sb, in_=wh["W2"][:])
            b1_sb = singles.tile([H, 1], F32)
            nc.sync.dma_start(out=b1_sb, in_=wh["b1"][:].unsqueeze(1))
            b2_sb = singles.tile([H, 1], F32)
            nc.sync.dma_start(out=b2_sb, in_=wh["b2"][:].unsqueeze(1))

            rhs_sb = singles.tile([H, NHEAD], F32)
            bias_row = singles.tile([1, NHEAD], F32)
            for w_name, b_name, c0, c1 in [
                ("Wk_r", "bk_r", KR0, SR0),
                ("Ws_r", "bs_r", SR0, KW0),
                ("Wk_w", "bk_w", KW0, SW0),
                ("Ws_w", "bs_w", SW0, E0),
                ("We_w", "be_w", E0, A0),
                ("Wa_w", "ba_w", A0, NHEAD),
            ]:
                nc.sync.dma_start(out=rhs_sb[:, c0:c1], in_=wh[w_name][:])
                nc.sync.dma_start(
                    out=bias_row[:, c0:c1], in_=wh[b_name][:].unsqueeze(0)
                )
            ones_sb = singles.tile([1, P], F32)
            nc.vector.memset(ones_sb, 1.0)

            # ---- per-tile pipeline ----
            # n_repeat>1 re-runs the whole loop (idempotent; bench-only)
            for t in range(NT * n_repeat):
                r0 = (t % NT) * P

                feats = ctl.tile([P, D + C], F32)
                nc.scalar.dma_start(out=feats[:, 0:D], in_=inputs_ap[r0:r0 + P, :])
                nc.scalar.dma_start(
                    out=feats[:, D:D + C].unsqueeze(1), in_=reading_ap[r0:r0 + P]
                )
                m = mem_pool.tile([P, L, C], F32)
                nc.sync.dma_start(out=m, in_=memory_ap[r0:r0 + P])

                # controller: h1 = tanh(feats@W1+b1); ctr = tanh(h1@W2+b2)
                t0 = ps_tr.tile([P, P], F32, tag="tr")
                nc.tensor.transpose(t0, feats[:, 0:128], ident)
                t1 = ps_tr.tile([P, P], F32, tag="tr")
                nc.tensor.transpose(t1, feats[:, 128:256], ident)
                t2 = ps_tr.tile([64, P], F32, tag="tr")
                nc.tensor.transpose(t2, feats[:, 256:320], ident)
                fT0 = ctl.tile([P, P], F32)
                nc.scalar.copy(fT0, t0)
                fT1 = ctl.tile([P, P], F32)
                nc.scalar.copy(fT1, t1)
                fT2 = ctl.tile([64, P], F32)
                nc.scalar.copy(fT2, t2)

                h1p = ps_mm.tile([H, P], F32, tag="mm")
                nc.tensor.matmul(h1p, lhsT=w1a, rhs=fT0, start=True, stop=False)
                nc.tensor.matmul(h1p, lhsT=w1b, rhs=fT1, start=False, stop=False)
                nc.tensor.matmul(h1p, lhsT=w1c, rhs=fT2, start=False, stop=True)
                h1T = ctl.tile([H, P], F32)
                nc.scalar.activation(h1T, h1p, AF.Tanh, bias=b1_sb)

                ctrp = ps_mm.tile([H, P], F32, tag="mm")
                nc.tensor.matmul(ctrp, lhsT=w2_sb, rhs=h1T, start=True, stop=True)
                ctrT = ctl.tile([H, P], F32)
                nc.scalar.activation(ctrT, ctrp, AF.Tanh, bias=b2_sb)

                # all six head projections; biases via 1-partition ones matmul
                hd = ps_hd.tile([P, NHEAD], F32)
                nc.tensor.matmul(hd, lhsT=ctrT, rhs=rhs_sb, start=True, stop=False)
                nc.tensor.matmul(hd, lhsT=ones_sb, rhs=bias_row, start=False, stop=True)

                # memory Frobenius norm^2 per batch row (ACT passes, halves)
                nsqa = bsc.tile([P, LH, C], F32, tag="bs")
                nm2a = ctl.tile([P, 1], F32)
                nc.scalar.activation(nsqa, m[:, 0:LH, :], AF.Square, accum_out=nm2a)
                nsqb = bsc.tile([P, LH, C], F32, tag="bs")
                nm2b = ctl.tile([P, 1], F32)
                nc.scalar.activation(nsqb, m[:, LH:L, :], AF.Square, accum_out=nm2b)
                nm2 = ctl.tile([P, 1], F32)
                nc.vector.tensor_add(nm2, nm2a, nm2b)
                nm = ctl.tile([P, 1], F32)
                nc.scalar.activation(nm, nm2, AF.Sqrt)
                nc.vector.tensor_scalar_max(nm, nm, EPS)

                # sigmoids grouped (single ACT table switch)
                s_r = ctl.tile([P, 1], F32)
                nc.scalar.activation(s_r, hd[:, SR0:SR0 + 1], AF.Sigmoid)
                s_w = ctl.tile([P, 1], F32)
                nc.scalar.activation(s_w, hd[:, SW0:SW0 + 1], AF.Sigmoid)
                e_sb = ctl.tile([P, C], F32)
                nc.scalar.activation(e_sb, hd[:, E0:A0], AF.Sigmoid)
                a_sb = ctl.tile([P, C], F32)
                nc.scalar.activation(a_sb, hd[:, A0:NHEAD], AF.Sigmoid)

                # per-head addressing: alpha = s / (max(|k|,eps)*max(|M|_F,eps))
                rsums = {}
                for name, k0, s_sig in (("r", KR0, s_r), ("w", KW0, s_w)):
                    k_ap = hd[:, k0:k0 + C]
                    ksq = ctl.tile([P, C], F32)
                    nk2 = ctl.tile([P, 1], F32)
                    nc.scalar.activation(ksq, k_ap, AF.Square, accum_out=nk2)
                    nk = ctl.tile([P, 1], F32)
                    nc.scalar.activation(nk, nk2, AF.Sqrt)
                    nc.vector.tensor_scalar_max(nk, nk, EPS)
                    den = ctl.tile([P, 1], F32)
                    nc.vector.tensor_mul(den, nk, nm)
                    rden = ctl.tile([P, 1], F32)
                    nc.vector.reciprocal(rden, den)
                    alpha = ctl.tile([P, 1], F32)
                    nc.vector.tensor_mul(alpha, s_sig, rden)

                    # logits: K[b,l] = alpha[b] * sum_c M[b,l,c]*k[b,c]
                    Kl = ctl.tile([P, L], F32)
                    kb = k_ap.unsqueeze(1).broadcast_to([P, LH, C])
                    for lh in range(2):
                        l0 = lh * LH
                        prod = bsc.tile([P, LH, C], F32, tag="bs")
                        nc.vector.scalar_tensor_tensor(
                            out=prod, in0=m[:, l0:l0 + LH, :], scalar=alpha,
                            in1=kb, op0=OP.mult, op1=OP.mult,
                        )
                        nc.vector.tensor_reduce(
                            Kl[:, l0:l0 + LH], prod, axis=AX.X, op=OP.add
                        )

                    # softmax over l: exp(K - max), sum via ACT accumulate
                    mxn = ctl.tile([P, 1], F32)
                    nc.vector.tensor_reduce(
                        mxn, Kl, axis=AX.X, op=OP.max, negate=True
                    )
                    ex = ctl.tile([P, L], F32)
                    sume = ctl.tile([P, 1], F32)
                    nc.scalar.activation(ex, Kl, AF.Exp, bias=mxn, accum_out=sume)
                    rsum = ctl.tile([P, 1], F32)
                    nc.vector.reciprocal(rsum, sume)
                    rsums[name] = (ex, rsum)

                # read output: read[b,c] = sum_l M[b,l,c] * w_r[b,l]
                ex_r, rsum_r = rsums["r"]
                rh = []
                for lh in range(2):
                    l0 = lh * LH
                    prodRT = bsc.tile([P, C, LH], F32, tag="bs")
                    mT = m[:, l0:l0 + LH, :].transpose([0, 2, 1])
                    exb = ex_r[:, l0:l0 + LH].unsqueeze(1).broadcast_to([P, C, LH])
                    nc.vector.scalar_tensor_tensor(
                        out=prodRT, in0=mT, scalar=rsum_r, in1=exb,
                        op0=OP.mult, op1=OP.mult,
                    )
                    rp = ctl.tile([P, C], F32, tag=f"rh{lh}")
                    nc.vector.tensor_reduce(rp, prodRT, axis=AX.X, op=OP.add)
                    rh.append(rp)
                read_sb = ctl.tile([P, C], F32)
                nc.vector.tensor_add(read_sb, rh[0], rh[1])
                nc.scalar.dma_start(out=read_ap[r0:r0 + P, :], in_=read_sb)

                # memory update: new_mem = M - M*(w_w x e) + (w_w x a)
                ex_w, rsum_w = rsums["w"]
                wwn = ctl.tile([P, L], F32)
                nc.vector.tensor_scalar_mul(wwn, ex_w, rsum_w)
                eb = e_sb.unsqueeze(1).broadcast_to([P, LH, C])
                ab = a_sb.unsqueeze(1).broadcast_to([P, LH, C])
                for lh in range(2):
                    l0 = lh * LH
                    msl = m[:, l0:l0 + LH, :]
                    wwb = wwn[:, l0:l0 + LH].unsqueeze(2).broadcast_to([P, LH, C])
                    Wh = wa.tile([P, LH, C], F32, tag="wa")
                    nc.gpsimd.tensor_tensor(out=Wh, in0=wwb, in1=eb, op=OP.mult)
                    Yh = bsc.tile([P, LH, C], F32, tag="bs")
                    nc.vector.scalar_tensor_tensor(
                        out=Yh, in0=Wh, scalar=1.0, in1=msl,
                        op0=OP.subtract, op1=OP.mult,
                    )  # (W-1)*M
                    Ah = wa.tile([P, LH, C], F32, tag="wa")
                    nc.gpsimd.tensor_tensor(out=Ah, in0=wwb, in1=ab, op=OP.mult)
                    Oh = outp.tile([P, LH, C], F32)
                    nc.vector.tensor_sub(Oh, Ah, Yh)  # A - (W-1)*M
                    eng = nc.sync if lh == 0 else nc.scalar
                    eng.dma_start(
                        out=newmem_ap[r0:r0 + P, l0:l0 + LH, :], in_=Oh
                    )

    return nc


_NC = {}


def _get_program(n_repeat=1):
    if n_repeat not in _NC:
        nc = _build_program(n_repeat)
        nc.finalize()
        _NC[n_repeat] = nc
    return _NC[n_repeat]


def _in_maps(inputs):
    weights = {name: np.ascontiguousarray(inputs[name], dtype=np.float32)
               for name, _ in WEIGHT_SPECS}
    in_maps = []
    for i in range(N_CORES):
        sl = slice(i * BC, (i + 1) * BC)
        in_maps.append({
            "inputs": np.ascontiguousarray(inputs["inputs"][sl], dtype=np.float32),
            "memory": np.ascontiguousarray(inputs["memory"][sl], dtype=np.float32),
            "reading": np.ascontiguousarray(inputs["reading"][sl], dtype=np.float32),
            **weights,
        })
    return in_maps


def _run(inputs, trace=False):
    nc = _get_program()
    return run_bass_kernel_spmd(nc, _in_maps(inputs), list(range(N_CORES)), trace=trace)


def kernel(**inputs):
    res = _run(inputs).results
    read = np.concatenate([res[i]["read_out"] for i in range(N_CORES)], axis=0)
    new_mem = np.concatenate([res[i]["new_mem"] for i in range(N_CORES)], axis=0)
    return read, new_mem
